# revision 15
# baseline (speedup 1.0000x reference)
"""AttentionDecoder step on 8 Trainium2 NeuronCores.

Sharding: batch-parallel attention (16 batches/core), tensor-parallel GRU
gates (128 E-rows/core), vocab-parallel output projection (4000 vocab/core).
Three AllGathers stitch the phases together on-device; log-softmax is
computed with a cross-core sum-of-exp (logits are small, so no max shift
is needed in fp32).
"""

import sys
import types

import numpy as np

# ---------------------------------------------------------------------------
# NTFF profile hook (missing antenv.axon_hooks on this image). Registering it
# is harmless when tracing is off and enables trace=True for profiling runs.
# ---------------------------------------------------------------------------
if "antenv.axon_hooks" not in sys.modules:
    _hooks = types.ModuleType("antenv.axon_hooks")
    _hooks._hook = None
    _hooks.set_axon_ntff_profile_hook = lambda h: setattr(_hooks, "_hook", h)
    _hooks.get_axon_ntff_profile_hook = lambda: _hooks._hook
    sys.modules["antenv.axon_hooks"] = _hooks
    try:
        import antenv

        antenv.axon_hooks = _hooks
        from trn_agent_boot.trn_boot import _ntff_profile_via_ctypes

        _hooks.set_axon_ntff_profile_hook(
            _ntff_profile_via_ctypes("/opt/axon/libaxon_pjrt.so")
        )
    except Exception:
        pass

import concourse.bacc as bacc
import concourse.masks as masks
import concourse.mybir as mybir
import concourse.tile as tile
from concourse.bass_utils import run_bass_kernel_spmd

B, S, E, V = 128, 256, 1024, 32000
NC = 8
BS = B // NC  # 16  batches per core
ES = E // NC  # 128 hidden rows per core (gate shard)
VS = V // NC  # 4000 vocab per core
E2 = 2 * E

f32 = mybir.dt.float32
f32r = mybir.dt.float32r

AF = mybir.ActivationFunctionType
RG = [list(range(NC))]

WARMUP_AG = True
DEBUG_TAPS = False


def r32(ap):
    """Reinterpret an f32 DRAM access pattern as f32r for matmul feeding."""
    return ap.bitcast(f32r)


def dma_kp_gather(nc, sb_ap, dram_ap, p=128, engine=None):
    """DMA dram [k*p, b] -> sbuf [p, k*b] (col = k*b + b_idx)."""
    kp, b = dram_ap.shape
    k = kp // p
    (engine or nc.sync).dma_start(
        sb_ap.rearrange("p (k b) -> p k b", k=k),
        dram_ap.rearrange("(k p) b -> p k b", p=p),
    )


def _build():
    nc = bacc.Bacc("TRN2", target_bir_lowering=False, debug=False, num_devices=NC)

    # ---------------- DRAM I/O ----------------
    def inp(name, shape):
        return nc.dram_tensor(name, shape, f32, kind="ExternalInput").ap()

    enc_s = inp("enc_s", [BS, S, E])
    x0T_b = inp("x0T_b", [E, BS])
    h0T_b = inp("h0T_b", [E, BS])
    h0T_full = inp("h0T_full", [E, B])
    h0_es = inp("h0_es", [B, ES])
    attn_W = inp("attn_W", [E2, S])
    attn_b = inp("attn_b", [1, S])
    comb_W = inp("comb_W", [E2, E])
    comb_b = inp("comb_b", [1, E])
    WihT_s = inp("WihT_s", [E, 3 * ES])
    WhhT_s = inp("WhhT_s", [E, 3 * ES])
    bihg_s = inp("bihg_s", [1, 3 * ES])
    bhhg_s = inp("bhhg_s", [1, 3 * ES])
    out_W_s = inp("out_W_s", [E, VS])
    out_b_s = inp("out_b_s", [1, VS])
    ones_in = inp("ones_in", [1, B])

    attn_w_out = nc.dram_tensor("attn_w_out", [BS, S], f32, kind="ExternalOutput").ap()
    h_newT_out = nc.dram_tensor("h_newT_out", [ES, B], f32, kind="ExternalOutput").ap()
    logp_out = nc.dram_tensor("logp_out", [B, VS], f32, kind="ExternalOutput").ap()
    if DEBUG_TAPS:
        cc_dbg = nc.dram_tensor("cc_dbg", [128, 16 * BS], f32, kind="ExternalOutput").ap()
        gru_dbg = nc.dram_tensor("gru_dbg", [BS, E], f32, kind="ExternalOutput").ap()
        gruT_dbg = nc.dram_tensor("gruT_dbg", [128, E], f32, kind="ExternalOutput").ap()
        rz_dbg = nc.dram_tensor("rz_dbg", [B, 2 * ES], f32, kind="ExternalOutput").ap()
        nn_dbg = nc.dram_tensor("nn_dbg", [B, ES], f32, kind="ExternalOutput").ap()

    # Collective bounce buffers
    ag1_in = nc.dram_tensor("ag1_in", [BS, E], f32)
    ag1_out = nc.dram_tensor("ag1_out", [B, E], f32, addr_space="Shared")
    ag2_in = nc.dram_tensor("ag2_in", [ES, B], f32)
    ag2_out = nc.dram_tensor("ag2_out", [E, B], f32, addr_space="Shared")
    ag3_in = nc.dram_tensor("ag3_in", [B, 1], f32)
    ag3_out = nc.dram_tensor("ag3_out", [NC * B, 1], f32, addr_space="Shared")
    if WARMUP_AG:
        wu_in = nc.dram_tensor("wu_in", [1, 1], f32)
        wu_out = nc.dram_tensor("wu_out", [NC, 1], f32, addr_space="Shared")

    with tile.TileContext(nc) as tc:
        with (
            tc.tile_pool(name="persist", bufs=1) as P,
            tc.tile_pool(name="aw_pool", bufs=2) as aw_pool,
            tc.tile_pool(name="enc_pool", bufs=4) as enc_pool,
            tc.tile_pool(name="cw_pool", bufs=2) as cw_pool,
            tc.tile_pool(name="wt_pool", bufs=2) as wt_pool,
            tc.tile_pool(name="ow_pool", bufs=2) as ow_pool,
            tc.tile_pool(name="small", bufs=2) as small,
        ):
            # ---- constants / biases ----
            id_sb = P.tile([128, 128], f32, name="id_sb")
            masks.make_identity(nc, id_sb[:])
            ones2 = P.tile([1, B], f32r, name="ones2")
            nc.sync.dma_start(ones2[:], r32(ones_in))
            ones1 = ones2

            attnb_sb = P.tile([1, S], f32r, name="attnb_sb")
            nc.sync.dma_start(attnb_sb[:], r32(attn_b))
            combb_sb = P.tile([1, E], f32r, name="combb_sb")
            nc.sync.dma_start(combb_sb[:], r32(comb_b))
            bihg_sb = P.tile([1, 3 * ES], f32r, name="bihg_sb")
            nc.sync.dma_start(bihg_sb[:], r32(bihg_s))
            bhhg_sb = P.tile([1, 3 * ES], f32r, name="bhhg_sb")
            nc.sync.dma_start(bhhg_sb[:], r32(bhhg_s))
            outb_sb = P.tile([1, VS], f32r, name="outb_sb")
            nc.gpsimd.dma_start(outb_sb[:], r32(out_b_s))

            # combinedT = [x0T_b ; h0T_b] packed as [128, 16*kt], kt in [0,16)
            comb_sb = P.tile([128, 16 * BS], f32r, name="comb_sb")
            dma_kp_gather(nc, comb_sb[:, 0 : 8 * BS], r32(x0T_b))
            dma_kp_gather(nc, comb_sb[:, 8 * BS : 16 * BS], r32(h0T_b))

            # ccT: attention part filled later; x0T part now
            cc_sb = P.tile([128, 16 * BS], f32r, name="cc_sb")
            dma_kp_gather(nc, cc_sb[:, 8 * BS : 16 * BS], r32(x0T_b))

            # h0T in [E-part-tiles, batch] layout: [128, k*128+b]
            h0T_sb = P.tile([128, E], f32r, name="h0T_sb")
            dma_kp_gather(nc, h0T_sb[:], r32(h0T_full), engine=nc.gpsimd)
            h0es_sb = P.tile([B, ES], f32, name="h0es_sb")
            nc.sync.dma_start(h0es_sb[:], h0_es)

            if WARMUP_AG:
                wu_sb = P.tile([1, 1], f32, name="wu_sb")
                nc.vector.memset(wu_sb[:], 0.0)
                nc.sync.dma_start(wu_in.ap(), wu_sb[:])
                nc.gpsimd.collective_compute(
                    "AllGather",
                    mybir.AluOpType.bypass,
                    replica_groups=RG,
                    ins=[wu_in.ap()],
                    outs=[wu_out.ap()],
                )

            with tc.tile_pool(name="psum1", bufs=1, space="PSUM") as psum1:
                # ================= scores + softmax =================
                ps_sc = psum1.tile([BS, S], f32, name="ps_sc", tag="sc")
                nc.tensor.matmul(ps_sc[:], ones1[:, 0:BS], attnb_sb[:], start=True, stop=False)
                for ktt in range(4):
                    awt = aw_pool.tile([128, 4 * S], f32r, name="awt")
                    nc.sync.dma_start(
                        awt[:].rearrange("p (k e) -> p k e", k=4),
                        r32(attn_W[ktt * 512 : (ktt + 1) * 512, :]).rearrange(
                            "(k p) e -> p k e", p=128
                        ),
                    )
                    for kk in range(4):
                        kt = 4 * ktt + kk
                        nc.tensor.matmul(
                            ps_sc[:],
                            comb_sb[:, kt * BS : (kt + 1) * BS],
                            awt[:, kk * S : (kk + 1) * S],
                            start=False,
                            stop=(kt == 15),
                        )
                # softmax over S (values are O(1): skip the max shift)
                exp_sb = small.tile([BS, S], f32, name="exp_sb")
                ssum = P.tile([BS, 1], f32, name="ssum")
                nc.scalar.activation(exp_sb[:], ps_sc[:], AF.Exp, accum_out=ssum[:])
                rinv = P.tile([BS, 1], f32, name="rinv")
                nc.vector.reciprocal(rinv[:], ssum[:])
                attn_w_sb = P.tile([BS, S], f32, name="attn_w_sb")
                nc.vector.tensor_scalar_mul(attn_w_sb[:], exp_sb[:], rinv[:])
                nc.sync.dma_start(attn_w_out, attn_w_sb[:])

                # attn_wT [128, kt*16+b] via PE transpose
                awT_sb = P.tile([128, 2 * BS], f32, name="awT_sb")
                for kt in range(2):
                    ps_tr = psum1.tile([128, 128], f32, name="ps_tr", tag="tr", bufs=1)
                    nc.tensor.transpose(
                        ps_tr[:, :BS],
                        attn_w_sb[:, kt * 128 : (kt + 1) * 128],
                        id_sb[:BS, :BS],
                    )
                    nc.scalar.activation(
                        awT_sb[:, kt * BS : (kt + 1) * BS], ps_tr[:, :BS], AF.Copy
                    )

                # ================= attention-applied =================
                # attn_applied [BS, E] = Wblk.T @ enc_flat, where Wblk
                # [B*S/..., BS] is the block-diagonal attention-weight matrix:
                # k-tile t covers (b = t//2, s-half = t%2); its only nonzero
                # column is b, holding awT's matching 128-row segment.
                wblk_sb = P.tile([128, 32 * BS], f32r, name="wblk_sb")
                zsc = small.tile([128, 32 * BS], f32, name="zsc")
                nc.vector.memset(zsc[:], 0.0)
                nc.vector.tensor_copy(wblk_sb[:], zsc[:])
                for t in range(32):
                    b, kt = t // 2, t % 2
                    nc.vector.tensor_copy(
                        wblk_sb[:, t * BS + b : t * BS + b + 1],
                        awT_sb[:, kt * BS + b : kt * BS + b + 1],
                    )
                ps_at = [
                    psum1.tile([BS, 512], f32, name=f"ps_at{c}", tag=f"attn{c}")
                    for c in range(2)
                ]
                enc_flat = enc_s.rearrange("b s e -> (b s) e")
                for tt in range(16):
                    et = enc_pool.tile([128, 2 * E], f32r, name="et")
                    nc.sync.dma_start(
                        et[:].rearrange("p (k e) -> p k e", k=2),
                        r32(enc_flat[tt * 256 : (tt + 1) * 256, :]).rearrange(
                            "(k p) e -> p k e", p=128
                        ),
                    )
                    for tk in range(2):
                        t = 2 * tt + tk
                        for c in range(2):
                            nc.tensor.matmul(
                                ps_at[c][:],
                                wblk_sb[:, t * BS : (t + 1) * BS],
                                et[:, tk * E + c * 512 : tk * E + (c + 1) * 512],
                                start=(t == 0),
                                stop=(t == 31),
                            )
                # -> att_sb [BS, E] then transpose into ccT tiles
                att_sb = small.tile([BS, E], f32, name="att_sb")
                for c in range(2):
                    nc.scalar.activation(
                        att_sb[:, c * 512 : (c + 1) * 512], ps_at[c][:], AF.Copy
                    )
                for mt in range(8):
                    ps_tr = psum1.tile([128, 128], f32, name="ps_tr", tag="tr", bufs=1)
                    nc.tensor.transpose(
                        ps_tr[:, :BS],
                        att_sb[:, mt * 128 : (mt + 1) * 128],
                        id_sb[:BS, :BS],
                    )
                    nc.scalar.activation(
                        cc_sb[:, mt * BS : (mt + 1) * BS], ps_tr[:, :BS], AF.Copy
                    )

                # ================= gru_in = ccT.T @ comb_W + comb_b =================
                pg = [
                    psum1.tile([BS, 512], f32, name=f"pg{c}", tag=f"pg{c}")
                    for c in range(2)
                ]
                for c in range(2):
                    nc.tensor.matmul(
                        pg[c][:],
                        ones1[:, 0:BS],
                        combb_sb[0:1, c * 512 : (c + 1) * 512],
                        start=True,
                        stop=False,
                    )
                for ktt in range(8):
                    cwt = cw_pool.tile([128, 2 * E], f32r, name="cwt")
                    nc.gpsimd.dma_start(
                        cwt[:].rearrange("p (k e) -> p k e", k=2),
                        r32(comb_W[ktt * 256 : (ktt + 1) * 256, :]).rearrange(
                            "(k p) e -> p k e", p=128
                        ),
                    )
                    for kk in range(2):
                        kt = 2 * ktt + kk
                        for c in range(2):
                            nc.tensor.matmul(
                                pg[c][:],
                                cc_sb[:, kt * BS : (kt + 1) * BS],
                                cwt[:, kk * E + c * 512 : kk * E + (c + 1) * 512],
                                start=False,
                                stop=(kt == 15),
                            )
                gru_sb = P.tile([BS, E], f32, name="gru_sb")
                for c in range(2):
                    nc.scalar.activation(
                        gru_sb[:, c * 512 : (c + 1) * 512], pg[c][:], AF.Copy
                    )
                nc.sync.dma_start(ag1_in.ap(), gru_sb[:])
                if DEBUG_TAPS:
                    nc.sync.dma_start(gru_dbg, gru_sb[:])
                    cc_f32 = small.tile([128, 16 * BS], f32, name="cc_f32")
                    nc.vector.tensor_copy(cc_f32[:], cc_sb[:].bitcast(f32))
                    nc.sync.dma_start(cc_dbg, cc_f32[:])

                # ================= AG1: gru_in -> full batch =================
                nc.gpsimd.collective_compute(
                    "AllGather",
                    mybir.AluOpType.bypass,
                    replica_groups=RG,
                    ins=[ag1_in.ap()],
                    outs=[ag1_out.ap()],
                )

                # gru_inT via PE transposes
                gruT_sb = P.tile([128, E], f32r, name="gruT_sb")
                for k in range(8):
                    gin = small.tile([B, 128], f32, name="gin")
                    nc.sync.dma_start(gin[:], ag1_out.ap()[:, k * 128 : (k + 1) * 128])
                    ps_tr = psum1.tile([128, 128], f32, name="ps_tr", tag="tr", bufs=1)
                    nc.tensor.transpose(ps_tr[:], gin[:], id_sb[:])
                    nc.scalar.activation(
                        gruT_sb[:, k * 128 : (k + 1) * 128], ps_tr[:], AF.Copy
                    )

                if DEBUG_TAPS:
                    gruT_f32 = small.tile([128, E], f32, name="gruT_f32")
                    nc.vector.tensor_copy(gruT_f32[:], gruT_sb[:].bitcast(f32))
                    nc.sync.dma_start(gruT_dbg, gruT_f32[:])
                # ================= gates (tensor-parallel over E) =================
                ps_rz = psum1.tile([B, 2 * ES], f32, name="ps_rz", tag="sc")
                ps_in = psum1.tile([B, ES], f32, name="ps_in", tag="attn0")
                ps_hn = psum1.tile([B, ES], f32, name="ps_hn", tag="attn1")
                nc.tensor.matmul(
                    ps_rz[:], ones2[:], bihg_sb[0:1, 0 : 2 * ES], start=True, stop=False
                )
                nc.tensor.matmul(
                    ps_rz[:], ones2[:], bhhg_sb[0:1, 0 : 2 * ES], start=False, stop=False
                )
                nc.tensor.matmul(
                    ps_in[:], ones2[:], bihg_sb[0:1, 2 * ES : 3 * ES], start=True, stop=False
                )
                nc.tensor.matmul(
                    ps_hn[:], ones2[:], bhhg_sb[0:1, 2 * ES : 3 * ES], start=True, stop=False
                )
                wih = P.tile([128, 8 * 3 * ES], f32r, name="wih")
                dma_kp_gather(nc, wih[:], r32(WihT_s), engine=nc.gpsimd)
                whh = P.tile([128, 8 * 3 * ES], f32r, name="whh")
                dma_kp_gather(nc, whh[:], r32(WhhT_s), engine=nc.gpsimd)
                G = 3 * ES
                for kt in range(8):
                    gslice = gruT_sb[:, kt * 128 : (kt + 1) * 128]
                    nc.tensor.matmul(
                        ps_rz[:],
                        gslice,
                        wih[:, kt * G : kt * G + 2 * ES],
                        start=False,
                        stop=False,
                    )
                    nc.tensor.matmul(
                        ps_in[:],
                        gslice,
                        wih[:, kt * G + 2 * ES : kt * G + 3 * ES],
                        start=False,
                        stop=(kt == 7),
                    )
                    hslice = h0T_sb[:, kt * 128 : (kt + 1) * 128]
                    nc.tensor.matmul(
                        ps_rz[:],
                        hslice,
                        whh[:, kt * G : kt * G + 2 * ES],
                        start=False,
                        stop=(kt == 7),
                    )
                    nc.tensor.matmul(
                        ps_hn[:],
                        hslice,
                        whh[:, kt * G + 2 * ES : kt * G + 3 * ES],
                        start=False,
                        stop=(kt == 7),
                    )
                rz_sb = small.tile([B, 2 * ES], f32, name="rz_sb")
                nc.scalar.activation(rz_sb[:], ps_rz[:], AF.Sigmoid)
                if DEBUG_TAPS:
                    nc.sync.dma_start(rz_dbg, rz_sb[:])
                t1 = small.tile([B, ES], f32, name="t1")
                nc.vector.tensor_mul(t1[:], rz_sb[:, 0:ES], ps_hn[:])
                t2 = small.tile([B, ES], f32, name="t2")
                nc.vector.tensor_add(t2[:], t1[:], ps_in[:])
                n_sb = small.tile([B, ES], f32, name="n_sb")
                nc.scalar.activation(n_sb[:], t2[:], AF.Tanh)
                if DEBUG_TAPS:
                    nc.sync.dma_start(nn_dbg, n_sb[:])
                d_sb = small.tile([B, ES], f32, name="d_sb")
                nc.vector.tensor_sub(d_sb[:], h0es_sb[:], n_sb[:])
                e_sb = small.tile([B, ES], f32, name="e_sb")
                nc.vector.tensor_mul(e_sb[:], rz_sb[:, ES : 2 * ES], d_sb[:])
                hnew_sb = small.tile([B, ES], f32, name="hnew_sb")
                nc.vector.tensor_add(hnew_sb[:], n_sb[:], e_sb[:])

                # transpose -> [ES, B], publish + AG2
                ps_tr = psum1.tile([128, 128], f32, name="ps_tr", tag="tr", bufs=1)
                nc.tensor.transpose(ps_tr[:], hnew_sb[:], id_sb[:])
                hnT_sb = small.tile([ES, B], f32, name="hnT_sb")
                nc.scalar.activation(hnT_sb[:], ps_tr[:], AF.Copy)
                nc.sync.dma_start(h_newT_out, hnT_sb[:])
                nc.sync.dma_start(ag2_in.ap(), hnT_sb[:])
                nc.gpsimd.collective_compute(
                    "AllGather",
                    mybir.AluOpType.bypass,
                    replica_groups=RG,
                    ins=[ag2_in.ap()],
                    outs=[ag2_out.ap()],
                )

            # ================= phase 2: out projection + log-softmax =============
            with tc.tile_pool(name="psum2", bufs=1, space="PSUM") as psum2:
                hT_sb = P.tile([128, E], f32r, name="hT_sb")
                dma_kp_gather(nc, hT_sb[:], r32(ag2_out.ap()))
                logits_sb = P.tile([B, VS], f32, name="logits_sb")
                sums_sb = P.tile([B, NC], f32, name="sums_sb")
                NCH = 8
                CH = VS // NCH  # 500
                pcs = [
                    psum2.tile([B, CH], f32, name=f"pc{c}", tag=f"pc{c}")
                    for c in range(NCH)
                ]
                for c in range(NCH):
                    nc.tensor.matmul(
                        pcs[c][:],
                        ones2[:],
                        outb_sb[0:1, c * CH : (c + 1) * CH],
                        start=True,
                        stop=False,
                    )
                for k in range(8):
                    owt = ow_pool.tile([128, VS], f32r, name="owt")
                    nc.scalar.dma_start(owt[:], r32(out_W_s[k * 128 : (k + 1) * 128, :]))
                    for c in range(NCH):
                        nc.tensor.matmul(
                            pcs[c][:],
                            hT_sb[:, k * 128 : (k + 1) * 128],
                            owt[:, c * CH : (c + 1) * CH],
                            start=False,
                            stop=(k == 7),
                        )
                for c in range(NCH):
                    nc.scalar.activation(
                        logits_sb[:, c * CH : (c + 1) * CH], pcs[c][:], AF.Copy
                    )
                    esc = small.tile([B, CH], f32, name="esc")
                    nc.scalar.activation(
                        esc[:], pcs[c][:], AF.Exp, accum_out=sums_sb[:, c : c + 1]
                    )
                lsum = P.tile([B, 1], f32, name="lsum")
                nc.vector.reduce_sum(lsum[:], sums_sb[:], axis=mybir.AxisListType.X)
                nc.sync.dma_start(ag3_in.ap(), lsum[:])
                nc.gpsimd.collective_compute(
                    "AllGather",
                    mybir.AluOpType.bypass,
                    replica_groups=RG,
                    ins=[ag3_in.ap()],
                    outs=[ag3_out.ap()],
                )
                s8_sb = P.tile([B, NC], f32, name="s8_sb")
                dma_kp_gather(nc, s8_sb[:], ag3_out.ap(), p=B)
                gsum = P.tile([B, 1], f32, name="gsum")
                nc.vector.reduce_sum(gsum[:], s8_sb[:], axis=mybir.AxisListType.X)
                logZ = P.tile([B, 1], f32, name="logZ")
                nc.scalar.activation(logZ[:], gsum[:], AF.Ln)
                nc.vector.tensor_scalar_sub(logits_sb[:], logits_sb[:], logZ[:])
                nc.sync.dma_start(logp_out, logits_sb[:])

    nc.finalize()
    return nc


_NC_CACHE = None


def _get_nc():
    global _NC_CACHE
    if _NC_CACHE is None:
        _NC_CACHE = _build()
    return _NC_CACHE


def _prep_in_maps(
    input_tensor,
    hidden_tensor,
    encoder_output,
    emb,
    attn_W,
    attn_b,
    comb_W,
    comb_b,
    W_ih,
    W_hh,
    b_ih,
    b_hh,
    out_W,
    out_b,
):
    f = np.float32
    idx = np.asarray(input_tensor).reshape(-1).astype(np.int64)
    emb = np.asarray(emb, f)
    x0 = emb[idx]  # [B, E]
    h0 = np.ascontiguousarray(np.asarray(hidden_tensor, f)[0])  # [B, E]
    x0T = np.ascontiguousarray(x0.T)
    h0T = np.ascontiguousarray(h0.T)
    enc = np.asarray(encoder_output, f)
    attn_W = np.ascontiguousarray(np.asarray(attn_W, f))
    attn_b = np.asarray(attn_b, f).reshape(1, S)
    comb_W = np.ascontiguousarray(np.asarray(comb_W, f))
    comb_b = np.asarray(comb_b, f).reshape(1, E)
    W_ih = np.asarray(W_ih, f)
    W_hh = np.asarray(W_hh, f)
    b_ih = np.asarray(b_ih, f)
    b_hh = np.asarray(b_hh, f)
    out_W = np.asarray(out_W, f)
    out_b = np.asarray(out_b, f)

    in_maps = []
    for i in range(NC):
        gate_rows = np.r_[
            i * ES : (i + 1) * ES,
            E + i * ES : E + (i + 1) * ES,
            2 * E + i * ES : 2 * E + (i + 1) * ES,
        ]
        in_maps.append(
            {
                "enc_s": np.ascontiguousarray(enc[i * BS : (i + 1) * BS]),
                "x0T_b": np.ascontiguousarray(x0T[:, i * BS : (i + 1) * BS]),
                "h0T_b": np.ascontiguousarray(h0T[:, i * BS : (i + 1) * BS]),
                "h0T_full": h0T,
                "h0_es": np.ascontiguousarray(h0[:, i * ES : (i + 1) * ES]),
                "attn_W": attn_W,
                "attn_b": attn_b,
                "comb_W": comb_W,
                "comb_b": comb_b,
                "WihT_s": np.ascontiguousarray(W_ih[gate_rows].T),
                "WhhT_s": np.ascontiguousarray(W_hh[gate_rows].T),
                "bihg_s": np.ascontiguousarray(b_ih[gate_rows].reshape(1, -1)),
                "bhhg_s": np.ascontiguousarray(b_hh[gate_rows].reshape(1, -1)),
                "out_W_s": np.ascontiguousarray(out_W[:, i * VS : (i + 1) * VS]),
                "out_b_s": np.ascontiguousarray(out_b[i * VS : (i + 1) * VS].reshape(1, -1)),
                "ones_in": np.ones((1, B), np.float32),
            }
        )
    return in_maps


def _assemble(results):
    log_probs = np.concatenate([r["logp_out"] for r in results], axis=1).reshape(
        B, 1, V
    )
    h_new = np.concatenate([r["h_newT_out"].T for r in results], axis=1)[None]
    attn_w = np.concatenate([r["attn_w_out"] for r in results], axis=0).reshape(
        B, 1, S
    )
    return log_probs, h_new, attn_w


def kernel_profiled(trace=False, **inputs):
    nc = _get_nc()
    in_maps = _prep_in_maps(**inputs)
    res = run_bass_kernel_spmd(nc, in_maps, core_ids=list(range(NC)), trace=trace)
    return _assemble(res.results), res


def kernel(**inputs):
    (log_probs, h_new, attn_w), _ = kernel_profiled(trace=False, **inputs)
    return log_probs, h_new, attn_w


# revision 16
# speedup vs baseline: 1.0903x; 1.0903x over previous
"""AttentionDecoder step on 8 Trainium2 NeuronCores.

Sharding: batch-parallel attention (16 batches/core), tensor-parallel GRU
gates (128 E-rows/core), vocab-parallel output projection (4000 vocab/core).
Three AllGathers stitch the phases together on-device; log-softmax is
computed with a cross-core sum-of-exp (logits are small, so no max shift
is needed in fp32).
"""

import sys
import types

import numpy as np

# ---------------------------------------------------------------------------
# NTFF profile hook (missing antenv.axon_hooks on this image). Registering it
# is harmless when tracing is off and enables trace=True for profiling runs.
# ---------------------------------------------------------------------------
if "antenv.axon_hooks" not in sys.modules:
    _hooks = types.ModuleType("antenv.axon_hooks")
    _hooks._hook = None
    _hooks.set_axon_ntff_profile_hook = lambda h: setattr(_hooks, "_hook", h)
    _hooks.get_axon_ntff_profile_hook = lambda: _hooks._hook
    sys.modules["antenv.axon_hooks"] = _hooks
    try:
        import antenv

        antenv.axon_hooks = _hooks
        from trn_agent_boot.trn_boot import _ntff_profile_via_ctypes

        _hooks.set_axon_ntff_profile_hook(
            _ntff_profile_via_ctypes("/opt/axon/libaxon_pjrt.so")
        )
    except Exception:
        pass

import concourse.bacc as bacc
import concourse.masks as masks
import concourse.mybir as mybir
import concourse.tile as tile
from concourse.bass_utils import run_bass_kernel_spmd

B, S, E, V = 128, 256, 1024, 32000
NC = 8
BS = B // NC  # 16  batches per core
ES = E // NC  # 128 hidden rows per core (gate shard)
VS = V // NC  # 4000 vocab per core
E2 = 2 * E

f32 = mybir.dt.float32
f32r = mybir.dt.float32r

AF = mybir.ActivationFunctionType
RG = [list(range(NC))]

WARMUP_AG = True
DEBUG_TAPS = False


def r32(ap):
    """Reinterpret an f32 DRAM access pattern as f32r for matmul feeding."""
    return ap.bitcast(f32r)


def dma_kp_gather(nc, sb_ap, dram_ap, p=128, engine=None):
    """DMA dram [k*p, b] -> sbuf [p, k*b] (col = k*b + b_idx)."""
    kp, b = dram_ap.shape
    k = kp // p
    (engine or nc.sync).dma_start(
        sb_ap.rearrange("p (k b) -> p k b", k=k),
        dram_ap.rearrange("(k p) b -> p k b", p=p),
    )


def _build():
    nc = bacc.Bacc("TRN2", target_bir_lowering=False, debug=False, num_devices=NC)

    # ---------------- DRAM I/O ----------------
    def inp(name, shape):
        return nc.dram_tensor(name, shape, f32, kind="ExternalInput").ap()

    enc_s = inp("enc_s", [BS, S, E])
    x0T_b = inp("x0T_b", [E, BS])
    h0T_b = inp("h0T_b", [E, BS])
    h0T_full = inp("h0T_full", [E, B])
    h0_es = inp("h0_es", [B, ES])
    attn_W = inp("attn_W", [E2, S])
    attn_b = inp("attn_b", [1, S])
    comb_W = inp("comb_W", [E2, E])
    comb_b = inp("comb_b", [1, E])
    WihT_s = inp("WihT_s", [E, 3 * ES])
    WhhT_s = inp("WhhT_s", [E, 3 * ES])
    bihg_s = inp("bihg_s", [1, 3 * ES])
    bhhg_s = inp("bhhg_s", [1, 3 * ES])
    out_W_s = inp("out_W_s", [E, VS])
    out_b_s = inp("out_b_s", [1, VS])
    ones_in = inp("ones_in", [1, B])

    attn_w_out = nc.dram_tensor("attn_w_out", [BS, S], f32, kind="ExternalOutput").ap()
    h_newT_out = nc.dram_tensor("h_newT_out", [ES, B], f32, kind="ExternalOutput").ap()
    logp_out = nc.dram_tensor("logp_out", [B, VS], f32, kind="ExternalOutput").ap()
    if DEBUG_TAPS:
        cc_dbg = nc.dram_tensor("cc_dbg", [128, 16 * BS], f32, kind="ExternalOutput").ap()
        gru_dbg = nc.dram_tensor("gru_dbg", [BS, E], f32, kind="ExternalOutput").ap()
        gruT_dbg = nc.dram_tensor("gruT_dbg", [128, E], f32, kind="ExternalOutput").ap()
        rz_dbg = nc.dram_tensor("rz_dbg", [B, 2 * ES], f32, kind="ExternalOutput").ap()
        nn_dbg = nc.dram_tensor("nn_dbg", [B, ES], f32, kind="ExternalOutput").ap()

    # Collective bounce buffers
    ag1_in = nc.dram_tensor("ag1_in", [BS, E], f32)
    ag1_out = nc.dram_tensor("ag1_out", [B, E], f32, addr_space="Shared")
    ag2_in = nc.dram_tensor("ag2_in", [ES, B], f32)
    ag2_out = nc.dram_tensor("ag2_out", [E, B], f32, addr_space="Shared")
    ag3_in = nc.dram_tensor("ag3_in", [B, 1], f32)
    ag3_out = nc.dram_tensor("ag3_out", [NC * B, 1], f32, addr_space="Shared")
    if WARMUP_AG:
        wu_in = nc.dram_tensor("wu_in", [1, 1], f32)
        wu_out = nc.dram_tensor("wu_out", [NC, 1], f32, addr_space="Shared")

    with tile.TileContext(nc) as tc:
        with (
            tc.tile_pool(name="persist", bufs=1) as P,
            tc.tile_pool(name="aw_pool", bufs=2) as aw_pool,
            tc.tile_pool(name="enc_pool", bufs=4) as enc_pool,
            tc.tile_pool(name="cw_pool", bufs=2) as cw_pool,
            tc.tile_pool(name="wt_pool", bufs=2) as wt_pool,
            tc.tile_pool(name="ow_pool", bufs=2) as ow_pool,
            tc.tile_pool(name="small", bufs=2) as small,
        ):
            # ---- constants / biases ----
            id_sb = P.tile([128, 128], f32, name="id_sb")
            masks.make_identity(nc, id_sb[:])
            ones2 = P.tile([1, B], f32r, name="ones2")
            nc.sync.dma_start(ones2[:], r32(ones_in))
            ones1 = ones2

            attnb_sb = P.tile([1, S], f32r, name="attnb_sb")
            nc.sync.dma_start(attnb_sb[:], r32(attn_b))
            combb_sb = P.tile([1, E], f32r, name="combb_sb")
            nc.sync.dma_start(combb_sb[:], r32(comb_b))
            bihg_sb = P.tile([1, 3 * ES], f32r, name="bihg_sb")
            nc.sync.dma_start(bihg_sb[:], r32(bihg_s))
            bhhg_sb = P.tile([1, 3 * ES], f32r, name="bhhg_sb")
            nc.sync.dma_start(bhhg_sb[:], r32(bhhg_s))
            outb_sb = P.tile([1, VS], f32r, name="outb_sb")
            nc.sync.dma_start(outb_sb[:], r32(out_b_s))

            # combinedT = [x0T_b ; h0T_b] packed as [128, 16*kt], kt in [0,16)
            comb_sb = P.tile([128, 16 * BS], f32r, name="comb_sb")
            dma_kp_gather(nc, comb_sb[:, 0 : 8 * BS], r32(x0T_b))
            dma_kp_gather(nc, comb_sb[:, 8 * BS : 16 * BS], r32(h0T_b))

            # ccT: attention part filled later; x0T part now
            cc_sb = P.tile([128, 16 * BS], f32r, name="cc_sb")
            dma_kp_gather(nc, cc_sb[:, 8 * BS : 16 * BS], r32(x0T_b))

            # h0T in [E-part-tiles, batch] layout: [128, k*128+b]
            h0T_sb = P.tile([128, E], f32r, name="h0T_sb")
            dma_kp_gather(nc, h0T_sb[:], r32(h0T_full))
            h0es_sb = P.tile([B, ES], f32, name="h0es_sb")
            nc.sync.dma_start(h0es_sb[:], h0_es)

            if WARMUP_AG:
                wu_sb = P.tile([1, 1], f32, name="wu_sb")
                nc.vector.memset(wu_sb[:], 0.0)
                nc.sync.dma_start(wu_in.ap(), wu_sb[:])
                nc.gpsimd.collective_compute(
                    "AllGather",
                    mybir.AluOpType.bypass,
                    replica_groups=RG,
                    ins=[wu_in.ap()],
                    outs=[wu_out.ap()],
                )

            with tc.tile_pool(name="psum1", bufs=1, space="PSUM") as psum1:
                # ================= scores + softmax =================
                ps_sc = psum1.tile([BS, S], f32, name="ps_sc", tag="sc")
                nc.tensor.matmul(ps_sc[:], ones1[:, 0:BS], attnb_sb[:], start=True, stop=False)
                for ktt in range(4):
                    awt = aw_pool.tile([128, 4 * S], f32r, name="awt")
                    nc.sync.dma_start(
                        awt[:].rearrange("p (k e) -> p k e", k=4),
                        r32(attn_W[ktt * 512 : (ktt + 1) * 512, :]).rearrange(
                            "(k p) e -> p k e", p=128
                        ),
                    )
                    for kk in range(4):
                        kt = 4 * ktt + kk
                        nc.tensor.matmul(
                            ps_sc[:],
                            comb_sb[:, kt * BS : (kt + 1) * BS],
                            awt[:, kk * S : (kk + 1) * S],
                            start=False,
                            stop=(kt == 15),
                        )
                # softmax over S (values are O(1): skip the max shift)
                exp_sb = small.tile([BS, S], f32, name="exp_sb")
                ssum = P.tile([BS, 1], f32, name="ssum")
                nc.scalar.activation(exp_sb[:], ps_sc[:], AF.Exp, accum_out=ssum[:])
                rinv = P.tile([BS, 1], f32, name="rinv")
                nc.vector.reciprocal(rinv[:], ssum[:])
                attn_w_sb = P.tile([BS, S], f32, name="attn_w_sb")
                nc.vector.tensor_scalar_mul(attn_w_sb[:], exp_sb[:], rinv[:])
                nc.sync.dma_start(attn_w_out, attn_w_sb[:])

                # attn_wT [128, kt*16+b] via PE transpose
                awT_sb = P.tile([128, 2 * BS], f32, name="awT_sb")
                for kt in range(2):
                    ps_tr = psum1.tile([128, 128], f32, name="ps_tr", tag="tr", bufs=1)
                    nc.tensor.transpose(
                        ps_tr[:, :BS],
                        attn_w_sb[:, kt * 128 : (kt + 1) * 128],
                        id_sb[:BS, :BS],
                    )
                    nc.scalar.activation(
                        awT_sb[:, kt * BS : (kt + 1) * BS], ps_tr[:, :BS], AF.Copy
                    )

                # ================= attention-applied =================
                # attn_applied [BS, E] = Wblk.T @ enc_flat, where Wblk
                # [B*S/..., BS] is the block-diagonal attention-weight matrix:
                # k-tile t covers (b = t//2, s-half = t%2); its only nonzero
                # column is b, holding awT's matching 128-row segment.
                wblk_sb = P.tile([128, 32 * BS], f32r, name="wblk_sb")
                zsc = small.tile([128, 32 * BS], f32, name="zsc")
                nc.vector.memset(zsc[:], 0.0)
                nc.vector.tensor_copy(wblk_sb[:], zsc[:])
                for t in range(32):
                    b, kt = t // 2, t % 2
                    nc.vector.tensor_copy(
                        wblk_sb[:, t * BS + b : t * BS + b + 1],
                        awT_sb[:, kt * BS + b : kt * BS + b + 1],
                    )
                ps_at = [
                    psum1.tile([BS, 512], f32, name=f"ps_at{c}", tag=f"attn{c}")
                    for c in range(2)
                ]
                enc_flat = enc_s.rearrange("b s e -> (b s) e")
                for tt in range(16):
                    et = enc_pool.tile([128, 2 * E], f32r, name="et")
                    nc.sync.dma_start(
                        et[:].rearrange("p (k e) -> p k e", k=2),
                        r32(enc_flat[tt * 256 : (tt + 1) * 256, :]).rearrange(
                            "(k p) e -> p k e", p=128
                        ),
                    )
                    for tk in range(2):
                        t = 2 * tt + tk
                        for c in range(2):
                            nc.tensor.matmul(
                                ps_at[c][:],
                                wblk_sb[:, t * BS : (t + 1) * BS],
                                et[:, tk * E + c * 512 : tk * E + (c + 1) * 512],
                                start=(t == 0),
                                stop=(t == 31),
                            )
                # -> att_sb [BS, E] then transpose into ccT tiles
                att_sb = small.tile([BS, E], f32, name="att_sb")
                for c in range(2):
                    nc.scalar.activation(
                        att_sb[:, c * 512 : (c + 1) * 512], ps_at[c][:], AF.Copy
                    )
                for mt in range(8):
                    ps_tr = psum1.tile([128, 128], f32, name="ps_tr", tag="tr", bufs=1)
                    nc.tensor.transpose(
                        ps_tr[:, :BS],
                        att_sb[:, mt * 128 : (mt + 1) * 128],
                        id_sb[:BS, :BS],
                    )
                    nc.scalar.activation(
                        cc_sb[:, mt * BS : (mt + 1) * BS], ps_tr[:, :BS], AF.Copy
                    )

                # ================= gru_in = ccT.T @ comb_W + comb_b =================
                pg = [
                    psum1.tile([BS, 512], f32, name=f"pg{c}", tag=f"pg{c}")
                    for c in range(2)
                ]
                for c in range(2):
                    nc.tensor.matmul(
                        pg[c][:],
                        ones1[:, 0:BS],
                        combb_sb[0:1, c * 512 : (c + 1) * 512],
                        start=True,
                        stop=False,
                    )
                for ktt in range(8):
                    cwt = cw_pool.tile([128, 2 * E], f32r, name="cwt")
                    nc.sync.dma_start(
                        cwt[:].rearrange("p (k e) -> p k e", k=2),
                        r32(comb_W[ktt * 256 : (ktt + 1) * 256, :]).rearrange(
                            "(k p) e -> p k e", p=128
                        ),
                    )
                    for kk in range(2):
                        kt = 2 * ktt + kk
                        for c in range(2):
                            nc.tensor.matmul(
                                pg[c][:],
                                cc_sb[:, kt * BS : (kt + 1) * BS],
                                cwt[:, kk * E + c * 512 : kk * E + (c + 1) * 512],
                                start=False,
                                stop=(kt == 15),
                            )
                gru_sb = P.tile([BS, E], f32, name="gru_sb")
                for c in range(2):
                    nc.scalar.activation(
                        gru_sb[:, c * 512 : (c + 1) * 512], pg[c][:], AF.Copy
                    )
                nc.sync.dma_start(ag1_in.ap(), gru_sb[:])
                if DEBUG_TAPS:
                    nc.sync.dma_start(gru_dbg, gru_sb[:])
                    cc_f32 = small.tile([128, 16 * BS], f32, name="cc_f32")
                    nc.vector.tensor_copy(cc_f32[:], cc_sb[:].bitcast(f32))
                    nc.sync.dma_start(cc_dbg, cc_f32[:])

                # ================= AG1: gru_in -> full batch =================
                nc.gpsimd.collective_compute(
                    "AllGather",
                    mybir.AluOpType.bypass,
                    replica_groups=RG,
                    ins=[ag1_in.ap()],
                    outs=[ag1_out.ap()],
                )

                # gru_inT via PE transposes
                gruT_sb = P.tile([128, E], f32r, name="gruT_sb")
                for k in range(8):
                    gin = small.tile([B, 128], f32, name="gin")
                    nc.sync.dma_start(gin[:], ag1_out.ap()[:, k * 128 : (k + 1) * 128])
                    ps_tr = psum1.tile([128, 128], f32, name="ps_tr", tag="tr", bufs=1)
                    nc.tensor.transpose(ps_tr[:], gin[:], id_sb[:])
                    nc.scalar.activation(
                        gruT_sb[:, k * 128 : (k + 1) * 128], ps_tr[:], AF.Copy
                    )

                if DEBUG_TAPS:
                    gruT_f32 = small.tile([128, E], f32, name="gruT_f32")
                    nc.vector.tensor_copy(gruT_f32[:], gruT_sb[:].bitcast(f32))
                    nc.sync.dma_start(gruT_dbg, gruT_f32[:])
                # ================= gates (tensor-parallel over E) =================
                ps_rz = psum1.tile([B, 2 * ES], f32, name="ps_rz", tag="sc")
                ps_in = psum1.tile([B, ES], f32, name="ps_in", tag="attn0")
                ps_hn = psum1.tile([B, ES], f32, name="ps_hn", tag="attn1")
                nc.tensor.matmul(
                    ps_rz[:], ones2[:], bihg_sb[0:1, 0 : 2 * ES], start=True, stop=False
                )
                nc.tensor.matmul(
                    ps_rz[:], ones2[:], bhhg_sb[0:1, 0 : 2 * ES], start=False, stop=False
                )
                nc.tensor.matmul(
                    ps_in[:], ones2[:], bihg_sb[0:1, 2 * ES : 3 * ES], start=True, stop=False
                )
                nc.tensor.matmul(
                    ps_hn[:], ones2[:], bhhg_sb[0:1, 2 * ES : 3 * ES], start=True, stop=False
                )
                wih = P.tile([128, 8 * 3 * ES], f32r, name="wih")
                dma_kp_gather(nc, wih[:], r32(WihT_s))
                whh = P.tile([128, 8 * 3 * ES], f32r, name="whh")
                dma_kp_gather(nc, whh[:], r32(WhhT_s))
                G = 3 * ES
                for kt in range(8):
                    gslice = gruT_sb[:, kt * 128 : (kt + 1) * 128]
                    nc.tensor.matmul(
                        ps_rz[:],
                        gslice,
                        wih[:, kt * G : kt * G + 2 * ES],
                        start=False,
                        stop=False,
                    )
                    nc.tensor.matmul(
                        ps_in[:],
                        gslice,
                        wih[:, kt * G + 2 * ES : kt * G + 3 * ES],
                        start=False,
                        stop=(kt == 7),
                    )
                    hslice = h0T_sb[:, kt * 128 : (kt + 1) * 128]
                    nc.tensor.matmul(
                        ps_rz[:],
                        hslice,
                        whh[:, kt * G : kt * G + 2 * ES],
                        start=False,
                        stop=(kt == 7),
                    )
                    nc.tensor.matmul(
                        ps_hn[:],
                        hslice,
                        whh[:, kt * G + 2 * ES : kt * G + 3 * ES],
                        start=False,
                        stop=(kt == 7),
                    )
                rz_sb = small.tile([B, 2 * ES], f32, name="rz_sb")
                nc.scalar.activation(rz_sb[:], ps_rz[:], AF.Sigmoid)
                if DEBUG_TAPS:
                    nc.sync.dma_start(rz_dbg, rz_sb[:])
                t1 = small.tile([B, ES], f32, name="t1")
                nc.vector.tensor_mul(t1[:], rz_sb[:, 0:ES], ps_hn[:])
                t2 = small.tile([B, ES], f32, name="t2")
                nc.vector.tensor_add(t2[:], t1[:], ps_in[:])
                n_sb = small.tile([B, ES], f32, name="n_sb")
                nc.scalar.activation(n_sb[:], t2[:], AF.Tanh)
                if DEBUG_TAPS:
                    nc.sync.dma_start(nn_dbg, n_sb[:])
                d_sb = small.tile([B, ES], f32, name="d_sb")
                nc.vector.tensor_sub(d_sb[:], h0es_sb[:], n_sb[:])
                e_sb = small.tile([B, ES], f32, name="e_sb")
                nc.vector.tensor_mul(e_sb[:], rz_sb[:, ES : 2 * ES], d_sb[:])
                hnew_sb = small.tile([B, ES], f32, name="hnew_sb")
                nc.vector.tensor_add(hnew_sb[:], n_sb[:], e_sb[:])

                # transpose -> [ES, B], publish + AG2
                ps_tr = psum1.tile([128, 128], f32, name="ps_tr", tag="tr", bufs=1)
                nc.tensor.transpose(ps_tr[:], hnew_sb[:], id_sb[:])
                hnT_sb = small.tile([ES, B], f32, name="hnT_sb")
                nc.scalar.activation(hnT_sb[:], ps_tr[:], AF.Copy)
                nc.sync.dma_start(h_newT_out, hnT_sb[:])
                nc.sync.dma_start(ag2_in.ap(), hnT_sb[:])
                nc.gpsimd.collective_compute(
                    "AllGather",
                    mybir.AluOpType.bypass,
                    replica_groups=RG,
                    ins=[ag2_in.ap()],
                    outs=[ag2_out.ap()],
                )

            # ================= phase 2: out projection + log-softmax =============
            with tc.tile_pool(name="psum2", bufs=1, space="PSUM") as psum2:
                hT_sb = P.tile([128, E], f32r, name="hT_sb")
                dma_kp_gather(nc, hT_sb[:], r32(ag2_out.ap()))
                logits_sb = P.tile([B, VS], f32, name="logits_sb")
                sums_sb = P.tile([B, NC], f32, name="sums_sb")
                NCH = 8
                CH = VS // NCH  # 500
                pcs = [
                    psum2.tile([B, CH], f32, name=f"pc{c}", tag=f"pc{c}")
                    for c in range(NCH)
                ]
                for c in range(NCH):
                    nc.tensor.matmul(
                        pcs[c][:],
                        ones2[:],
                        outb_sb[0:1, c * CH : (c + 1) * CH],
                        start=True,
                        stop=False,
                    )
                for k in range(8):
                    owt = ow_pool.tile([128, VS], f32r, name="owt")
                    nc.scalar.dma_start(owt[:], r32(out_W_s[k * 128 : (k + 1) * 128, :]))
                    for c in range(NCH):
                        nc.tensor.matmul(
                            pcs[c][:],
                            hT_sb[:, k * 128 : (k + 1) * 128],
                            owt[:, c * CH : (c + 1) * CH],
                            start=False,
                            stop=(k == 7),
                        )
                for c in range(NCH):
                    nc.scalar.activation(
                        logits_sb[:, c * CH : (c + 1) * CH], pcs[c][:], AF.Copy
                    )
                    esc = small.tile([B, CH], f32, name="esc")
                    nc.scalar.activation(
                        esc[:], pcs[c][:], AF.Exp, accum_out=sums_sb[:, c : c + 1]
                    )
                lsum = P.tile([B, 1], f32, name="lsum")
                nc.vector.reduce_sum(lsum[:], sums_sb[:], axis=mybir.AxisListType.X)
                nc.sync.dma_start(ag3_in.ap(), lsum[:])
                nc.gpsimd.collective_compute(
                    "AllGather",
                    mybir.AluOpType.bypass,
                    replica_groups=RG,
                    ins=[ag3_in.ap()],
                    outs=[ag3_out.ap()],
                )
                s8_sb = P.tile([B, NC], f32, name="s8_sb")
                dma_kp_gather(nc, s8_sb[:], ag3_out.ap(), p=B)
                gsum = P.tile([B, 1], f32, name="gsum")
                nc.vector.reduce_sum(gsum[:], s8_sb[:], axis=mybir.AxisListType.X)
                logZ = P.tile([B, 1], f32, name="logZ")
                nc.scalar.activation(logZ[:], gsum[:], AF.Ln)
                nc.vector.tensor_scalar_sub(logits_sb[:], logits_sb[:], logZ[:])
                nc.sync.dma_start(logp_out, logits_sb[:])

    nc.finalize()
    return nc


_NC_CACHE = None


def _get_nc():
    global _NC_CACHE
    if _NC_CACHE is None:
        _NC_CACHE = _build()
    return _NC_CACHE


def _prep_in_maps(
    input_tensor,
    hidden_tensor,
    encoder_output,
    emb,
    attn_W,
    attn_b,
    comb_W,
    comb_b,
    W_ih,
    W_hh,
    b_ih,
    b_hh,
    out_W,
    out_b,
):
    f = np.float32
    idx = np.asarray(input_tensor).reshape(-1).astype(np.int64)
    emb = np.asarray(emb, f)
    x0 = emb[idx]  # [B, E]
    h0 = np.ascontiguousarray(np.asarray(hidden_tensor, f)[0])  # [B, E]
    x0T = np.ascontiguousarray(x0.T)
    h0T = np.ascontiguousarray(h0.T)
    enc = np.asarray(encoder_output, f)
    attn_W = np.ascontiguousarray(np.asarray(attn_W, f))
    attn_b = np.asarray(attn_b, f).reshape(1, S)
    comb_W = np.ascontiguousarray(np.asarray(comb_W, f))
    comb_b = np.asarray(comb_b, f).reshape(1, E)
    W_ih = np.asarray(W_ih, f)
    W_hh = np.asarray(W_hh, f)
    b_ih = np.asarray(b_ih, f)
    b_hh = np.asarray(b_hh, f)
    out_W = np.asarray(out_W, f)
    out_b = np.asarray(out_b, f)

    in_maps = []
    for i in range(NC):
        gate_rows = np.r_[
            i * ES : (i + 1) * ES,
            E + i * ES : E + (i + 1) * ES,
            2 * E + i * ES : 2 * E + (i + 1) * ES,
        ]
        in_maps.append(
            {
                "enc_s": np.ascontiguousarray(enc[i * BS : (i + 1) * BS]),
                "x0T_b": np.ascontiguousarray(x0T[:, i * BS : (i + 1) * BS]),
                "h0T_b": np.ascontiguousarray(h0T[:, i * BS : (i + 1) * BS]),
                "h0T_full": h0T,
                "h0_es": np.ascontiguousarray(h0[:, i * ES : (i + 1) * ES]),
                "attn_W": attn_W,
                "attn_b": attn_b,
                "comb_W": comb_W,
                "comb_b": comb_b,
                "WihT_s": np.ascontiguousarray(W_ih[gate_rows].T),
                "WhhT_s": np.ascontiguousarray(W_hh[gate_rows].T),
                "bihg_s": np.ascontiguousarray(b_ih[gate_rows].reshape(1, -1)),
                "bhhg_s": np.ascontiguousarray(b_hh[gate_rows].reshape(1, -1)),
                "out_W_s": np.ascontiguousarray(out_W[:, i * VS : (i + 1) * VS]),
                "out_b_s": np.ascontiguousarray(out_b[i * VS : (i + 1) * VS].reshape(1, -1)),
                "ones_in": np.ones((1, B), np.float32),
            }
        )
    return in_maps


def _assemble(results):
    log_probs = np.concatenate([r["logp_out"] for r in results], axis=1).reshape(
        B, 1, V
    )
    h_new = np.concatenate([r["h_newT_out"].T for r in results], axis=1)[None]
    attn_w = np.concatenate([r["attn_w_out"] for r in results], axis=0).reshape(
        B, 1, S
    )
    return log_probs, h_new, attn_w


def kernel_profiled(trace=False, **inputs):
    nc = _get_nc()
    in_maps = _prep_in_maps(**inputs)
    res = run_bass_kernel_spmd(nc, in_maps, core_ids=list(range(NC)), trace=trace)
    return _assemble(res.results), res


def kernel(**inputs):
    (log_probs, h_new, attn_w), _ = kernel_profiled(trace=False, **inputs)
    return log_probs, h_new, attn_w


# revision 17
# speedup vs baseline: 1.1854x; 1.0872x over previous
"""AttentionDecoder step on 8 Trainium2 NeuronCores.

Sharding: batch-parallel attention (16 batches/core), tensor-parallel GRU
gates (128 E-rows/core), vocab-parallel output projection (4000 vocab/core).
Three AllGathers stitch the phases together on-device; log-softmax is
computed with a cross-core sum-of-exp (logits are small, so no max shift
is needed in fp32).
"""

import sys
import types

import numpy as np

# ---------------------------------------------------------------------------
# NTFF profile hook (missing antenv.axon_hooks on this image). Registering it
# is harmless when tracing is off and enables trace=True for profiling runs.
# ---------------------------------------------------------------------------
if "antenv.axon_hooks" not in sys.modules:
    _hooks = types.ModuleType("antenv.axon_hooks")
    _hooks._hook = None
    _hooks.set_axon_ntff_profile_hook = lambda h: setattr(_hooks, "_hook", h)
    _hooks.get_axon_ntff_profile_hook = lambda: _hooks._hook
    sys.modules["antenv.axon_hooks"] = _hooks
    try:
        import antenv

        antenv.axon_hooks = _hooks
        from trn_agent_boot.trn_boot import _ntff_profile_via_ctypes

        _hooks.set_axon_ntff_profile_hook(
            _ntff_profile_via_ctypes("/opt/axon/libaxon_pjrt.so")
        )
    except Exception:
        pass

import concourse.bacc as bacc
import concourse.masks as masks
import concourse.mybir as mybir
import concourse.tile as tile
from concourse.bass_utils import run_bass_kernel_spmd

B, S, E, V = 128, 256, 1024, 32000
NC = 8
BS = B // NC  # 16  batches per core
ES = E // NC  # 128 hidden rows per core (gate shard)
VS = V // NC  # 4000 vocab per core
E2 = 2 * E

f32 = mybir.dt.float32
f32r = mybir.dt.float32r

AF = mybir.ActivationFunctionType
RG = [list(range(NC))]

WARMUP_AG = True
DEBUG_TAPS = False


def r32(ap):
    """Reinterpret an f32 DRAM access pattern as f32r for matmul feeding."""
    return ap.bitcast(f32r)


def dma_kp_gather(nc, sb_ap, dram_ap, p=128, engine=None):
    """DMA dram [k*p, b] -> sbuf [p, k*b] (col = k*b + b_idx)."""
    kp, b = dram_ap.shape
    k = kp // p
    (engine or nc.sync).dma_start(
        sb_ap.rearrange("p (k b) -> p k b", k=k),
        dram_ap.rearrange("(k p) b -> p k b", p=p),
    )


def _build():
    nc = bacc.Bacc("TRN2", target_bir_lowering=False, debug=False, num_devices=NC)

    # ---------------- DRAM I/O ----------------
    def inp(name, shape):
        return nc.dram_tensor(name, shape, f32, kind="ExternalInput").ap()

    enc_s = inp("enc_s", [BS, S, E])
    x0T_b = inp("x0T_b", [E, BS])
    h0T_b = inp("h0T_b", [E, BS])
    h0T_full = inp("h0T_full", [E, B])
    h0_es = inp("h0_es", [B, ES])
    attn_W = inp("attn_W", [E2, S])
    attn_b = inp("attn_b", [1, S])
    comb_W = inp("comb_W", [E2, E])
    comb_b = inp("comb_b", [1, E])
    WihT_s = inp("WihT_s", [E, 3 * ES])
    WhhT_s = inp("WhhT_s", [E, 3 * ES])
    bihg_s = inp("bihg_s", [1, 3 * ES])
    bhhg_s = inp("bhhg_s", [1, 3 * ES])
    out_W_s = inp("out_W_s", [E, VS])
    out_b_s = inp("out_b_s", [1, VS])
    ones_in = inp("ones_in", [1, B])

    attn_w_out = nc.dram_tensor("attn_w_out", [BS, S], f32, kind="ExternalOutput").ap()
    h_newT_out = nc.dram_tensor("h_newT_out", [ES, B], f32, kind="ExternalOutput").ap()
    logp_out = nc.dram_tensor("logp_out", [B, VS], f32, kind="ExternalOutput").ap()
    if DEBUG_TAPS:
        cc_dbg = nc.dram_tensor("cc_dbg", [128, 16 * BS], f32, kind="ExternalOutput").ap()
        gru_dbg = nc.dram_tensor("gru_dbg", [BS, E], f32, kind="ExternalOutput").ap()
        gruT_dbg = nc.dram_tensor("gruT_dbg", [128, E], f32, kind="ExternalOutput").ap()
        rz_dbg = nc.dram_tensor("rz_dbg", [B, 2 * ES], f32, kind="ExternalOutput").ap()
        nn_dbg = nc.dram_tensor("nn_dbg", [B, ES], f32, kind="ExternalOutput").ap()

    # Collective bounce buffers
    ag1_in = nc.dram_tensor("ag1_in", [BS, E], f32)
    ag1_out = nc.dram_tensor("ag1_out", [B, E], f32, addr_space="Shared")
    ag2_in = nc.dram_tensor("ag2_in", [ES, B], f32)
    ag2_out = nc.dram_tensor("ag2_out", [E, B], f32, addr_space="Shared")
    ag3_in = nc.dram_tensor("ag3_in", [B, 1], f32)
    ag3_out = nc.dram_tensor("ag3_out", [NC * B, 1], f32, addr_space="Shared")
    if WARMUP_AG:
        wu_in = nc.dram_tensor("wu_in", [1, 1], f32)
        wu_out = nc.dram_tensor("wu_out", [NC, 1], f32, addr_space="Shared")

    with tile.TileContext(nc) as tc:
        with (
            tc.tile_pool(name="persist", bufs=1) as P,
            tc.tile_pool(name="small", bufs=2) as small,
        ):
            # ---- constants / biases ----
            id_sb = P.tile([128, 128], f32, name="id_sb")
            masks.make_identity(nc, id_sb[:])
            ones2 = P.tile([1, B], f32r, name="ones2")
            nc.sync.dma_start(ones2[:], r32(ones_in))
            ones1 = ones2

            attnb_sb = P.tile([1, S], f32r, name="attnb_sb")
            nc.sync.dma_start(attnb_sb[:], r32(attn_b))
            combb_sb = P.tile([1, E], f32r, name="combb_sb")
            nc.sync.dma_start(combb_sb[:], r32(comb_b))
            bihg_sb = P.tile([1, 3 * ES], f32r, name="bihg_sb")
            nc.sync.dma_start(bihg_sb[:], r32(bihg_s))
            bhhg_sb = P.tile([1, 3 * ES], f32r, name="bhhg_sb")
            nc.sync.dma_start(bhhg_sb[:], r32(bhhg_s))
            outb_sb = P.tile([1, VS], f32r, name="outb_sb")
            nc.sync.dma_start(outb_sb[:], r32(out_b_s))

            # combinedT = [x0T_b ; h0T_b] packed as [128, 16*kt], kt in [0,16)
            comb_sb = P.tile([128, 16 * BS], f32r, name="comb_sb")
            dma_kp_gather(nc, comb_sb[:, 0 : 8 * BS], r32(x0T_b))
            dma_kp_gather(nc, comb_sb[:, 8 * BS : 16 * BS], r32(h0T_b))

            # ccT: attention part filled later; x0T part now
            cc_sb = P.tile([128, 16 * BS], f32r, name="cc_sb")
            dma_kp_gather(nc, cc_sb[:, 8 * BS : 16 * BS], r32(x0T_b))

            # h0T in [E-part-tiles, batch] layout: [128, k*128+b]
            h0T_sb = P.tile([128, E], f32r, name="h0T_sb")
            dma_kp_gather(nc, h0T_sb[:], r32(h0T_full))
            h0es_sb = P.tile([B, ES], f32, name="h0es_sb")
            nc.sync.dma_start(h0es_sb[:], h0_es)

            if WARMUP_AG:
                wu_sb = P.tile([1, 1], f32, name="wu_sb")
                nc.vector.memset(wu_sb[:], 0.0)
                nc.sync.dma_start(wu_in.ap(), wu_sb[:])
                nc.gpsimd.collective_compute(
                    "AllGather",
                    mybir.AluOpType.bypass,
                    replica_groups=RG,
                    ins=[wu_in.ap()],
                    outs=[wu_out.ap()],
                )

            with (
                tc.tile_pool(name="aw_pool", bufs=2) as aw_pool,
                tc.tile_pool(name="enc_pool", bufs=4) as enc_pool,
                tc.tile_pool(name="cw_pool", bufs=2) as cw_pool,
                tc.tile_pool(name="psum1", bufs=1, space="PSUM") as psum1,
            ):
                # ================= scores + softmax =================
                ps_sc = psum1.tile([BS, S], f32, name="ps_sc", tag="sc")
                nc.tensor.matmul(ps_sc[:], ones1[:, 0:BS], attnb_sb[:], start=True, stop=False)
                for ktt in range(4):
                    awt = aw_pool.tile([128, 4 * S], f32r, name="awt")
                    nc.sync.dma_start(
                        awt[:].rearrange("p (k e) -> p k e", k=4),
                        r32(attn_W[ktt * 512 : (ktt + 1) * 512, :]).rearrange(
                            "(k p) e -> p k e", p=128
                        ),
                    )
                    for kk in range(4):
                        kt = 4 * ktt + kk
                        nc.tensor.matmul(
                            ps_sc[:],
                            comb_sb[:, kt * BS : (kt + 1) * BS],
                            awt[:, kk * S : (kk + 1) * S],
                            start=False,
                            stop=(kt == 15),
                        )
                # softmax over S (values are O(1): skip the max shift)
                exp_sb = small.tile([BS, S], f32, name="exp_sb")
                ssum = P.tile([BS, 1], f32, name="ssum")
                nc.scalar.activation(exp_sb[:], ps_sc[:], AF.Exp, accum_out=ssum[:])
                rinv = P.tile([BS, 1], f32, name="rinv")
                nc.vector.reciprocal(rinv[:], ssum[:])
                attn_w_sb = P.tile([BS, S], f32, name="attn_w_sb")
                nc.vector.tensor_scalar_mul(attn_w_sb[:], exp_sb[:], rinv[:])
                nc.sync.dma_start(attn_w_out, attn_w_sb[:])

                # attn_wT [128, kt*16+b] via PE transpose
                awT_sb = P.tile([128, 2 * BS], f32, name="awT_sb")
                for kt in range(2):
                    ps_tr = psum1.tile([128, 128], f32, name="ps_tr", tag="tr", bufs=1)
                    nc.tensor.transpose(
                        ps_tr[:, :BS],
                        attn_w_sb[:, kt * 128 : (kt + 1) * 128],
                        id_sb[:BS, :BS],
                    )
                    nc.scalar.activation(
                        awT_sb[:, kt * BS : (kt + 1) * BS], ps_tr[:, :BS], AF.Copy
                    )

                # ================= attention-applied =================
                # attn_applied [BS, E] = Wblk.T @ enc_flat, where Wblk
                # [B*S/..., BS] is the block-diagonal attention-weight matrix:
                # k-tile t covers (b = t//2, s-half = t%2); its only nonzero
                # column is b, holding awT's matching 128-row segment.
                wblk_sb = P.tile([128, 32 * BS], f32r, name="wblk_sb")
                zsc = small.tile([128, 32 * BS], f32, name="zsc")
                nc.vector.memset(zsc[:], 0.0)
                nc.vector.tensor_copy(wblk_sb[:], zsc[:])
                for t in range(32):
                    b, kt = t // 2, t % 2
                    nc.vector.tensor_copy(
                        wblk_sb[:, t * BS + b : t * BS + b + 1],
                        awT_sb[:, kt * BS + b : kt * BS + b + 1],
                    )
                ps_at = [
                    psum1.tile([BS, 512], f32, name=f"ps_at{c}", tag=f"attn{c}")
                    for c in range(2)
                ]
                enc_flat = enc_s.rearrange("b s e -> (b s) e")
                for tt in range(16):
                    et = enc_pool.tile([128, 2 * E], f32r, name="et")
                    nc.sync.dma_start(
                        et[:].rearrange("p (k e) -> p k e", k=2),
                        r32(enc_flat[tt * 256 : (tt + 1) * 256, :]).rearrange(
                            "(k p) e -> p k e", p=128
                        ),
                    )
                    for tk in range(2):
                        t = 2 * tt + tk
                        for c in range(2):
                            nc.tensor.matmul(
                                ps_at[c][:],
                                wblk_sb[:, t * BS : (t + 1) * BS],
                                et[:, tk * E + c * 512 : tk * E + (c + 1) * 512],
                                start=(t == 0),
                                stop=(t == 31),
                            )
                # -> att_sb [BS, E] then transpose into ccT tiles
                att_sb = small.tile([BS, E], f32, name="att_sb")
                for c in range(2):
                    nc.scalar.activation(
                        att_sb[:, c * 512 : (c + 1) * 512], ps_at[c][:], AF.Copy
                    )
                for mt in range(8):
                    ps_tr = psum1.tile([128, 128], f32, name="ps_tr", tag="tr", bufs=1)
                    nc.tensor.transpose(
                        ps_tr[:, :BS],
                        att_sb[:, mt * 128 : (mt + 1) * 128],
                        id_sb[:BS, :BS],
                    )
                    nc.scalar.activation(
                        cc_sb[:, mt * BS : (mt + 1) * BS], ps_tr[:, :BS], AF.Copy
                    )

                # ================= gru_in = ccT.T @ comb_W + comb_b =================
                pg = [
                    psum1.tile([BS, 512], f32, name=f"pg{c}", tag=f"pg{c}")
                    for c in range(2)
                ]
                for c in range(2):
                    nc.tensor.matmul(
                        pg[c][:],
                        ones1[:, 0:BS],
                        combb_sb[0:1, c * 512 : (c + 1) * 512],
                        start=True,
                        stop=False,
                    )
                for ktt in range(8):
                    cwt = cw_pool.tile([128, 2 * E], f32r, name="cwt")
                    nc.sync.dma_start(
                        cwt[:].rearrange("p (k e) -> p k e", k=2),
                        r32(comb_W[ktt * 256 : (ktt + 1) * 256, :]).rearrange(
                            "(k p) e -> p k e", p=128
                        ),
                    )
                    for kk in range(2):
                        kt = 2 * ktt + kk
                        for c in range(2):
                            nc.tensor.matmul(
                                pg[c][:],
                                cc_sb[:, kt * BS : (kt + 1) * BS],
                                cwt[:, kk * E + c * 512 : kk * E + (c + 1) * 512],
                                start=False,
                                stop=(kt == 15),
                            )
                gru_sb = P.tile([BS, E], f32, name="gru_sb")
                for c in range(2):
                    nc.scalar.activation(
                        gru_sb[:, c * 512 : (c + 1) * 512], pg[c][:], AF.Copy
                    )
                nc.sync.dma_start(ag1_in.ap(), gru_sb[:])
                if DEBUG_TAPS:
                    nc.sync.dma_start(gru_dbg, gru_sb[:])
                    cc_f32 = small.tile([128, 16 * BS], f32, name="cc_f32")
                    nc.vector.tensor_copy(cc_f32[:], cc_sb[:].bitcast(f32))
                    nc.sync.dma_start(cc_dbg, cc_f32[:])

                # ================= AG1: gru_in -> full batch =================
                nc.gpsimd.collective_compute(
                    "AllGather",
                    mybir.AluOpType.bypass,
                    replica_groups=RG,
                    ins=[ag1_in.ap()],
                    outs=[ag1_out.ap()],
                )

                # gru_inT via PE transposes
                gruT_sb = P.tile([128, E], f32r, name="gruT_sb")
                for k in range(8):
                    gin = small.tile([B, 128], f32, name="gin")
                    nc.sync.dma_start(gin[:], ag1_out.ap()[:, k * 128 : (k + 1) * 128])
                    ps_tr = psum1.tile([128, 128], f32, name="ps_tr", tag="tr", bufs=1)
                    nc.tensor.transpose(ps_tr[:], gin[:], id_sb[:])
                    nc.scalar.activation(
                        gruT_sb[:, k * 128 : (k + 1) * 128], ps_tr[:], AF.Copy
                    )

                if DEBUG_TAPS:
                    gruT_f32 = small.tile([128, E], f32, name="gruT_f32")
                    nc.vector.tensor_copy(gruT_f32[:], gruT_sb[:].bitcast(f32))
                    nc.sync.dma_start(gruT_dbg, gruT_f32[:])
                # ================= gates (tensor-parallel over E) =================
                ps_rz = psum1.tile([B, 2 * ES], f32, name="ps_rz", tag="sc")
                ps_in = psum1.tile([B, ES], f32, name="ps_in", tag="attn0")
                ps_hn = psum1.tile([B, ES], f32, name="ps_hn", tag="attn1")
                nc.tensor.matmul(
                    ps_rz[:], ones2[:], bihg_sb[0:1, 0 : 2 * ES], start=True, stop=False
                )
                nc.tensor.matmul(
                    ps_rz[:], ones2[:], bhhg_sb[0:1, 0 : 2 * ES], start=False, stop=False
                )
                nc.tensor.matmul(
                    ps_in[:], ones2[:], bihg_sb[0:1, 2 * ES : 3 * ES], start=True, stop=False
                )
                nc.tensor.matmul(
                    ps_hn[:], ones2[:], bhhg_sb[0:1, 2 * ES : 3 * ES], start=True, stop=False
                )
                wih = P.tile([128, 8 * 3 * ES], f32r, name="wih")
                dma_kp_gather(nc, wih[:], r32(WihT_s))
                whh = P.tile([128, 8 * 3 * ES], f32r, name="whh")
                dma_kp_gather(nc, whh[:], r32(WhhT_s))
                G = 3 * ES
                for kt in range(8):
                    gslice = gruT_sb[:, kt * 128 : (kt + 1) * 128]
                    nc.tensor.matmul(
                        ps_rz[:],
                        gslice,
                        wih[:, kt * G : kt * G + 2 * ES],
                        start=False,
                        stop=False,
                    )
                    nc.tensor.matmul(
                        ps_in[:],
                        gslice,
                        wih[:, kt * G + 2 * ES : kt * G + 3 * ES],
                        start=False,
                        stop=(kt == 7),
                    )
                    hslice = h0T_sb[:, kt * 128 : (kt + 1) * 128]
                    nc.tensor.matmul(
                        ps_rz[:],
                        hslice,
                        whh[:, kt * G : kt * G + 2 * ES],
                        start=False,
                        stop=(kt == 7),
                    )
                    nc.tensor.matmul(
                        ps_hn[:],
                        hslice,
                        whh[:, kt * G + 2 * ES : kt * G + 3 * ES],
                        start=False,
                        stop=(kt == 7),
                    )
                rz_sb = small.tile([B, 2 * ES], f32, name="rz_sb")
                nc.scalar.activation(rz_sb[:], ps_rz[:], AF.Sigmoid)
                if DEBUG_TAPS:
                    nc.sync.dma_start(rz_dbg, rz_sb[:])
                t1 = small.tile([B, ES], f32, name="t1")
                nc.vector.tensor_mul(t1[:], rz_sb[:, 0:ES], ps_hn[:])
                t2 = small.tile([B, ES], f32, name="t2")
                nc.vector.tensor_add(t2[:], t1[:], ps_in[:])
                n_sb = small.tile([B, ES], f32, name="n_sb")
                nc.scalar.activation(n_sb[:], t2[:], AF.Tanh)
                if DEBUG_TAPS:
                    nc.sync.dma_start(nn_dbg, n_sb[:])
                d_sb = small.tile([B, ES], f32, name="d_sb")
                nc.vector.tensor_sub(d_sb[:], h0es_sb[:], n_sb[:])
                e_sb = small.tile([B, ES], f32, name="e_sb")
                nc.vector.tensor_mul(e_sb[:], rz_sb[:, ES : 2 * ES], d_sb[:])
                hnew_sb = small.tile([B, ES], f32, name="hnew_sb")
                nc.vector.tensor_add(hnew_sb[:], n_sb[:], e_sb[:])

                # transpose -> [ES, B], publish + AG2
                ps_tr = psum1.tile([128, 128], f32, name="ps_tr", tag="tr", bufs=1)
                nc.tensor.transpose(ps_tr[:], hnew_sb[:], id_sb[:])
                hnT_sb = small.tile([ES, B], f32, name="hnT_sb")
                nc.scalar.activation(hnT_sb[:], ps_tr[:], AF.Copy)
                nc.sync.dma_start(h_newT_out, hnT_sb[:])
                nc.sync.dma_start(ag2_in.ap(), hnT_sb[:])
                nc.gpsimd.collective_compute(
                    "AllGather",
                    mybir.AluOpType.bypass,
                    replica_groups=RG,
                    ins=[ag2_in.ap()],
                    outs=[ag2_out.ap()],
                )

            # ================= phase 2: out projection + log-softmax =============
            with (
                tc.tile_pool(name="ow_pool", bufs=5) as ow_pool,
                tc.tile_pool(name="psum2", bufs=1, space="PSUM") as psum2,
            ):
                hT_sb = P.tile([128, E], f32r, name="hT_sb")
                dma_kp_gather(nc, hT_sb[:], r32(ag2_out.ap()))
                logits_sb = P.tile([B, VS], f32, name="logits_sb")
                sums_sb = P.tile([B, NC], f32, name="sums_sb")
                NCH = 8
                CH = VS // NCH  # 500
                pcs = [
                    psum2.tile([B, CH], f32, name=f"pc{c}", tag=f"pc{c}")
                    for c in range(NCH)
                ]
                for c in range(NCH):
                    nc.tensor.matmul(
                        pcs[c][:],
                        ones2[:],
                        outb_sb[0:1, c * CH : (c + 1) * CH],
                        start=True,
                        stop=False,
                    )
                for k in range(8):
                    owt = ow_pool.tile([128, VS], f32r, name="owt")
                    nc.sync.dma_start(owt[:], r32(out_W_s[k * 128 : (k + 1) * 128, :]))
                    for c in range(NCH):
                        nc.tensor.matmul(
                            pcs[c][:],
                            hT_sb[:, k * 128 : (k + 1) * 128],
                            owt[:, c * CH : (c + 1) * CH],
                            start=False,
                            stop=(k == 7),
                        )
                for c in range(NCH):
                    nc.scalar.activation(
                        logits_sb[:, c * CH : (c + 1) * CH], pcs[c][:], AF.Copy
                    )
                    esc = small.tile([B, CH], f32, name="esc")
                    nc.scalar.activation(
                        esc[:], pcs[c][:], AF.Exp, accum_out=sums_sb[:, c : c + 1]
                    )
                lsum = P.tile([B, 1], f32, name="lsum")
                nc.vector.reduce_sum(lsum[:], sums_sb[:], axis=mybir.AxisListType.X)
                nc.sync.dma_start(ag3_in.ap(), lsum[:])
                nc.gpsimd.collective_compute(
                    "AllGather",
                    mybir.AluOpType.bypass,
                    replica_groups=RG,
                    ins=[ag3_in.ap()],
                    outs=[ag3_out.ap()],
                )
                s8_sb = P.tile([B, NC], f32, name="s8_sb")
                dma_kp_gather(nc, s8_sb[:], ag3_out.ap(), p=B)
                gsum = P.tile([B, 1], f32, name="gsum")
                nc.vector.reduce_sum(gsum[:], s8_sb[:], axis=mybir.AxisListType.X)
                logZ = P.tile([B, 1], f32, name="logZ")
                nc.scalar.activation(logZ[:], gsum[:], AF.Ln)
                nc.vector.tensor_scalar_sub(logits_sb[:], logits_sb[:], logZ[:])
                nc.sync.dma_start(logp_out, logits_sb[:])

    nc.finalize()
    return nc


_NC_CACHE = None


def _get_nc():
    global _NC_CACHE
    if _NC_CACHE is None:
        _NC_CACHE = _build()
    return _NC_CACHE


def _prep_in_maps(
    input_tensor,
    hidden_tensor,
    encoder_output,
    emb,
    attn_W,
    attn_b,
    comb_W,
    comb_b,
    W_ih,
    W_hh,
    b_ih,
    b_hh,
    out_W,
    out_b,
):
    f = np.float32
    idx = np.asarray(input_tensor).reshape(-1).astype(np.int64)
    emb = np.asarray(emb, f)
    x0 = emb[idx]  # [B, E]
    h0 = np.ascontiguousarray(np.asarray(hidden_tensor, f)[0])  # [B, E]
    x0T = np.ascontiguousarray(x0.T)
    h0T = np.ascontiguousarray(h0.T)
    enc = np.asarray(encoder_output, f)
    attn_W = np.ascontiguousarray(np.asarray(attn_W, f))
    attn_b = np.asarray(attn_b, f).reshape(1, S)
    comb_W = np.ascontiguousarray(np.asarray(comb_W, f))
    comb_b = np.asarray(comb_b, f).reshape(1, E)
    W_ih = np.asarray(W_ih, f)
    W_hh = np.asarray(W_hh, f)
    b_ih = np.asarray(b_ih, f)
    b_hh = np.asarray(b_hh, f)
    out_W = np.asarray(out_W, f)
    out_b = np.asarray(out_b, f)

    in_maps = []
    for i in range(NC):
        gate_rows = np.r_[
            i * ES : (i + 1) * ES,
            E + i * ES : E + (i + 1) * ES,
            2 * E + i * ES : 2 * E + (i + 1) * ES,
        ]
        in_maps.append(
            {
                "enc_s": np.ascontiguousarray(enc[i * BS : (i + 1) * BS]),
                "x0T_b": np.ascontiguousarray(x0T[:, i * BS : (i + 1) * BS]),
                "h0T_b": np.ascontiguousarray(h0T[:, i * BS : (i + 1) * BS]),
                "h0T_full": h0T,
                "h0_es": np.ascontiguousarray(h0[:, i * ES : (i + 1) * ES]),
                "attn_W": attn_W,
                "attn_b": attn_b,
                "comb_W": comb_W,
                "comb_b": comb_b,
                "WihT_s": np.ascontiguousarray(W_ih[gate_rows].T),
                "WhhT_s": np.ascontiguousarray(W_hh[gate_rows].T),
                "bihg_s": np.ascontiguousarray(b_ih[gate_rows].reshape(1, -1)),
                "bhhg_s": np.ascontiguousarray(b_hh[gate_rows].reshape(1, -1)),
                "out_W_s": np.ascontiguousarray(out_W[:, i * VS : (i + 1) * VS]),
                "out_b_s": np.ascontiguousarray(out_b[i * VS : (i + 1) * VS].reshape(1, -1)),
                "ones_in": np.ones((1, B), np.float32),
            }
        )
    return in_maps


def _assemble(results):
    log_probs = np.concatenate([r["logp_out"] for r in results], axis=1).reshape(
        B, 1, V
    )
    h_new = np.concatenate([r["h_newT_out"].T for r in results], axis=1)[None]
    attn_w = np.concatenate([r["attn_w_out"] for r in results], axis=0).reshape(
        B, 1, S
    )
    return log_probs, h_new, attn_w


def kernel_profiled(trace=False, **inputs):
    nc = _get_nc()
    in_maps = _prep_in_maps(**inputs)
    res = run_bass_kernel_spmd(nc, in_maps, core_ids=list(range(NC)), trace=trace)
    return _assemble(res.results), res


def kernel(**inputs):
    (log_probs, h_new, attn_w), _ = kernel_profiled(trace=False, **inputs)
    return log_probs, h_new, attn_w
